# revision 30
# baseline (speedup 1.0000x reference)
"""Trainium2 Bass kernel for a 2-layer relational GNN actor (edge MLPs per
edge type, mean aggregation by destination node, node MLP, tanh head).

Sharding: nodes are split into 8 contiguous blocks of 6250; each core owns the
edges whose dst falls in its block (host-side bucketing).  Per core, edges are
grouped by (dst window of 128 nodes, etype) and padded to 128-edge chunks so
the whole edge phase is a static instruction schedule shared by all cores.

Math (per layer, exploiting linearity of segment-sum):
  h1[e]  = Ps_t[src[e]] + Pd_t[dst[e]] + (ef[e] @ W1e_t + b1_t)   (t = etype[e])
  m[e]   = relu(h1[e]) * escale[e],   escale[e] = 1 / cnt(dst[e], t)
  agg_t[n] = sum_{e: dst=n,type=t} m[e]      (= mean of relu(h1))
  z_t[n]   = sum escale = [cnt>0]
  node in = [nf | agg_0 | z_0 | agg_1 | z_1] with the edge-MLP second layer
  (W2_t, b2_t) folded into the node-MLP first-layer weights on the host.

On device: Ps/Pd projection tables are built per layer ([4*N, 64] in DRAM) and
rows are fetched per edge chunk with indirect DMA gathers; segment-sum is a
selection-matrix matmul (S[e, j] = dst_local[e] == j) accumulating into PSUM
per (window, etype).
"""

import numpy as np

from concourse import bass, mybir
import concourse.tile as tile
from concourse.bass_utils import run_bass_kernel_spmd
from concourse.vector_clock import ScopedClock, VectorClock


# --- workarounds for a walrus build that allows at most ONE sync-wait per
# instruction: split the TileContext tail drain's waits across SP nops, and
# post-process the final module hoisting excess waits onto injected no-ops.

def _drain_and_barrier_chunked(self, tick_clock, wait_clock):
    nc = self.nc
    gclock = tick_clock.global_clock
    for lane in range(len(gclock)):
        tick = gclock[lane]
        if tick <= 0:
            continue
        vec = [0] * len(gclock)
        vec[lane] = tick
        nop_inst = nc.sync.nop(nofuse=True, hint=f"drain_wait_l{lane}")
        wait_clock.add_sem_waits(nop_inst.ins, ScopedClock({None: VectorClock(vec)}))
    nc.sync.drain()
    nc.all_engine_barrier()
    assert self.sems is not None
    popped = nc._tile_sem_poison_stack.pop()
    assert popped is self._sem_poison
    nc.clear_and_free_semaphores(list(self.sems.allocated().values()))
    nc.all_engine_barrier()


tile.TileContext._drain_and_barrier = _drain_and_barrier_chunked

_nop_counter = [0]


def _split_multiwaits(nc):
    for fn in nc.m.functions:
        for blk in fn.blocks:
            new_insts = []
            changed = False
            for inst in blk.instructions:
                si = inst.sync_info
                if si is not None and len(si.on_wait) > 1:
                    waits = list(si.on_wait)
                    for w in waits[:-1]:
                        _nop_counter[0] += 1
                        nop = mybir.InstNoOp(
                            name=f"I-waitsplit-{_nop_counter[0]}",
                            engine=inst.engine,
                            sync_info=mybir.SyncInfo(on_wait=[w], on_update=[]),
                        )
                        new_insts.append(nop)
                    si.on_wait = [waits[-1]]
                    changed = True
                new_insts.append(inst)
            if changed:
                blk.instructions = new_insts

F32 = mybir.dt.float32
BF16 = mybir.dt.bfloat16
I32 = mybir.dt.int32
AF = mybir.ActivationFunctionType

N = 50000
E = 800000
NODE_IN = 32
EDGE_IN = 16
H = 64
AD = 8
NT = 2
NCORES = 8
B = N // NCORES          # 6250 nodes per core
WIN = 128                # scatter window (nodes)
NWIN = (B + WIN - 1) // WIN   # 49
PADLOC = 300.0           # dst_local for padding edges (never matches iota)
TROWS = 4 * N            # projection table rows: src t0|src t1|dst t0|dst t1
GROUP = 4                # chunks per batched group (512 edges)
IBLK = 64                # chunks per idx/scalar stream load
EBLK = 16                # chunks per efta stream load
NCHUNK_N = (N + 127) // 128   # 391 node chunks


def _prep(nf, ef, src, dst, etype, params):
    """Host-side preprocessing: edge schedule, streams, folded weights."""
    nf = np.asarray(nf, np.float32)
    ef = np.asarray(ef, np.float32)
    src = np.asarray(src).astype(np.int32)
    dst = np.asarray(dst).astype(np.int32)
    etype = np.asarray(etype).astype(np.int32)

    core = dst // B
    win = (dst % B) // WIN

    order = np.lexsort((dst, etype, win, core))
    so_core, so_win, so_t = core[order], win[order], etype[order]
    key = (so_core.astype(np.int64) * NWIN + so_win) * NT + so_t
    cnts = np.bincount(key, minlength=NCORES * NWIN * NT).reshape(NCORES, NWIN, NT)
    chunks_per = (cnts + 127) // 128
    C = np.maximum(chunks_per.max(axis=0), 1)  # shared schedule [NWIN, NT]
    nch = int(C.sum())
    if nch % GROUP:
        C[NWIN - 1, NT - 1] += GROUP - nch % GROUP
        nch = int(C.sum())
    esched = nch * 128

    sched = []
    for w in range(NWIN):
        for t in range(NT):
            sched.extend([(w, t)] * int(C[w, t]))

    cnt_nt = np.bincount(dst * NT + etype, minlength=N * NT).astype(np.float32)

    run_starts = np.zeros(NCORES * NWIN * NT + 1, np.int64)
    np.cumsum(cnts.reshape(-1), out=run_starts[1:])

    per_core = []
    for c in range(NCORES):
        gsrc = np.zeros(esched, np.int32)
        dloc = np.full(esched, PADLOC, np.float32)
        escl = np.zeros(esched, np.float32)
        efTa = np.zeros((EDGE_IN + 1, esched), np.float32)
        efTa[EDGE_IN] = 1.0
        pos = 0
        for w in range(NWIN):
            for t in range(NT):
                ridx = (c * NWIN + w) * NT + t
                s0, s1 = run_starts[ridx], run_starts[ridx + 1]
                eidx = order[s0:s1]
                k = int(s1 - s0)
                gsrc[pos:pos + k] = src[eidx]
                dloc[pos:pos + k] = (dst[eidx] % B) % WIN
                escl[pos:pos + k] = 1.0 / cnt_nt[dst[eidx] * NT + etype[eidx]]
                efTa[:EDGE_IN, pos:pos + k] = ef[eidx].T
                pos += int(C[w, t]) * 128
        per_core.append(dict(
            gsrc_cols=np.ascontiguousarray(gsrc.reshape(nch, 128).T),
            dloc_cols=np.ascontiguousarray(dloc.reshape(nch, 128).T),
            dloc_row=np.ascontiguousarray(dloc.reshape(1, esched)),
            escl_cols=np.ascontiguousarray(escl.reshape(nch, 128).T),
            efta=np.ascontiguousarray(efTa),
            nft_blk=np.ascontiguousarray(nf[c * B:(c + 1) * B].T),
        ))

    import ml_dtypes
    bf16 = ml_dtypes.bfloat16
    for pc in per_core:
        pc["efta"] = pc["efta"].astype(bf16)
        pc["dloc_row"] = pc["dloc_row"].astype(bf16)
    nft0 = np.ascontiguousarray(nf.T)  # [32, N]
    iotag = np.tile(np.arange(128, dtype=np.float32), (128, GROUP))  # [128, 512]
    iotap = np.tile(np.arange(128, dtype=np.float32)[:, None], (1, 128 * GROUP))
    ones1 = np.ones((1, 128), bf16)

    layers = params["layers"]
    wts = {}
    for l, layer in enumerate(layers):
        nin = NODE_IN if l == 0 else H
        w1 = [np.asarray(layer["edge"][t][0][0], np.float32) for t in range(NT)]
        b1 = [np.asarray(layer["edge"][t][0][1], np.float32) for t in range(NT)]
        w2 = [np.asarray(layer["edge"][t][1][0], np.float32) for t in range(NT)]
        b2 = [np.asarray(layer["edge"][t][1][1], np.float32) for t in range(NT)]
        wn1, bn1 = (np.asarray(a, np.float32) for a in layer["node"][0])
        wn2, bn2 = (np.asarray(a, np.float32) for a in layer["node"][1])
        wsd = np.concatenate([w1[0][:nin], w1[1][:nin]], axis=1)
        wdd = np.concatenate([w1[0][nin:2 * nin], w1[1][nin:2 * nin]], axis=1)
        wef = []
        for t in range(NT):
            m = np.zeros((EDGE_IN + 1, H + 1), np.float32)
            m[:EDGE_IN, :H] = w1[t][2 * nin:]
            m[EDGE_IN, :H] = b1[t]
            m[EDGE_IN, H] = 1.0
            wef.append(m)
        wn_nf = wn1[:nin]
        wn_a = []
        for t in range(NT):
            blk = wn1[nin + t * H: nin + (t + 1) * H]
            m = np.zeros((H + 1, H), np.float32)
            m[:H] = w2[t] @ blk
            m[H] = b2[t] @ blk
            wn_a.append(m)
        wts[l] = dict(wsd=wsd, wdd=wdd,
                      wef=[m.astype(bf16) for m in wef],
                      wn_nf=wn_nf,
                      wn_a=[m.astype(bf16) for m in wn_a],
                      bn1=bn1.reshape(H, 1), wn2=wn2.astype(bf16),
                      bn2=bn2.reshape(H, 1))
    wts["head"] = [(np.asarray(w, np.float32).astype(bf16),
                    np.asarray(b, np.float32).reshape(-1, 1))
                   for w, b in params["head"]]
    return per_core, sched, nch, esched, nft0, (iotag, iotap, ones1), wts


def _build(sched, nch, esched, wts):
    nc = bass.Bass(num_devices=NCORES, dynamic_dma_scratch_size=65536)
    NIN = {0: NODE_IN, 1: H}
    hd = wts["head"]

    ext = {}

    def dram_in(name, shape, dt=F32):
        ext[name] = nc.declare_dram_parameter(name, list(shape), dt, isOutput=False)
        return ext[name]

    dram_in("nft0", [NODE_IN, N])
    dram_in("nft_blk0", [NODE_IN, B])
    dram_in("iotag", [128, 128 * GROUP])
    dram_in("iotap", [128, 128 * GROUP])
    dram_in("ones1", [1, 128], BF16)
    dram_in("gsrc_cols", [128, nch], I32)
    dram_in("dloc_cols", [128, nch])
    dram_in("dloc_row", [1, esched], BF16)
    dram_in("escl_cols", [128, nch])
    dram_in("efta", [EDGE_IN + 1, esched], BF16)
    for l in (0, 1):
        nin = NIN[l]
        dram_in(f"wsd{l}", [nin, 2 * H])
        dram_in(f"wdd{l}", [nin, 2 * H])
        for t in range(NT):
            dram_in(f"wef{l}_{t}", [EDGE_IN + 1, H + 1], BF16)
        dram_in(f"wn_nf{l}", [nin, H])
        for t in range(NT):
            dram_in(f"wn_a{l}_{t}", [H + 1, H], BF16)
        dram_in(f"bn1_{l}", [H, 1])
        dram_in(f"wn2_{l}", [H, H], BF16)
        dram_in(f"bn2_{l}", [H, 1])
    for i, (w, b) in enumerate(hd):
        dram_in(f"wh{i}", w.shape, BF16)
        dram_in(f"bh{i}", b.shape)
    out_ext = nc.declare_dram_parameter("out", [AD, B], F32, isOutput=True)

    with tile.TileContext(nc, num_cores=NCORES) as tc:
        with (
            tc.tile_pool(name="const", bufs=1) as cpool,
            tc.tile_pool(name="blk", bufs=1) as bpool,
            tc.tile_pool(name="stream", bufs=4) as spool,
            tc.tile_pool(name="efs", bufs=2) as efpool,
            tc.tile_pool(name="nfs", bufs=2) as nfspool,
            tc.tile_pool(name="work", bufs=3) as wpool,
            tc.tile_pool(name="ps_proj", bufs=2, space="PSUM") as ps_proj,
            tc.tile_pool(name="ps_ef", bufs=2, space="PSUM") as ps_ef,
            tc.tile_pool(name="ps_agg", bufs=2, space="PSUM") as ps_agg,
            tc.tile_pool(name="ps_node", bufs=1, space="PSUM") as ps_node,
            tc.tile_pool(name="ps_bc", bufs=1, space="PSUM") as ps_bc,
            tc.tile_pool(name="dram", bufs=1, space="DRAM") as dpool,
        ):
            def load(name, shape, dt=F32, pool=cpool):
                t_ = pool.tile(shape, dt, tag=name, name=name)
                nc.sync.dma_start(out=t_[:], in_=ext[name][:])
                return t_

            iotag = load("iotag", [128, 128 * GROUP])
            iotap = load("iotap", [128, 128 * GROUP])
            ones1 = load("ones1", [1, 128], BF16)
            W = {}
            for l in (0, 1):
                nin = NIN[l]
                W[f"wsd{l}"] = load(f"wsd{l}", [nin, 2 * H])
                W[f"wdd{l}"] = load(f"wdd{l}", [nin, 2 * H])
                for t in range(NT):
                    W[f"wef{l}_{t}"] = load(f"wef{l}_{t}", [EDGE_IN + 1, H + 1], BF16)
                W[f"wn_nf{l}"] = load(f"wn_nf{l}", [nin, H])
                for t in range(NT):
                    W[f"wn_a{l}_{t}"] = load(f"wn_a{l}_{t}", [H + 1, H], BF16)
                W[f"bn1_{l}"] = load(f"bn1_{l}", [H, 1])
                W[f"wn2_{l}"] = load(f"wn2_{l}", [H, H], BF16)
                W[f"bn2_{l}"] = load(f"bn2_{l}", [H, 1])
            for i, (w, b) in enumerate(hd):
                W[f"wh{i}"] = load(f"wh{i}", list(w.shape), BF16)
                W[f"bh{i}"] = load(f"bh{i}", list(b.shape))

            nb0 = bpool.tile([NODE_IN, B], F32, tag="nftblk02", name="nftblk0")
            nc.sync.dma_start(out=nb0[:], in_=ext["nft_blk0"][:])
            nft_blk = {0: nb0}
            nft_blk[1] = bpool.tile([H, B], F32, tag="nftblk1", name="nftblk1")
            nft_blk[2] = bpool.tile([H, B], BF16, tag="nftblk02", name="nftblk2")

            T = dpool.tile([N, 2 * H], F32)
            agbounce = dpool.tile([H, B], F32)
            agout = nc.dram_tensor("agout_sh", [NCORES, H, B], F32,
                                   addr_space="Shared")
            nft1d = dpool.tile([H, N], F32)
            nft_src = {0: ext["nft0"], 1: nft1d}

            def rg(ap, inner):
                return ap.rearrange("p (g c) -> p g c", c=inner)

            for l in (0, 1):
                nin = NIN[l]
                # ---- phase A: projection tables into DRAM ----
                NFB = 16  # node chunks per streamed nft block
                TB = 4    # chunks batched per table-write DMA
                NFULL = (NCHUNK_N // TB) * TB if N % 128 else NCHUNK_N
                stage = None
                for ch in range(NCHUNK_N):
                    if ch % NFB == 0:
                        ncols_blk = min(NFB * 128, N - ch * 128)
                        nfs = nfspool.tile([nin, NFB * 128], F32, tag="nfs")
                        nc.sync.dma_start(out=nfs[:, :ncols_blk],
                                          in_=nft_src[l][:, ch * 128: ch * 128 + ncols_blk])
                    cb = (ch % NFB) * 128
                    n0 = ch * 128
                    ncols = min(128, N - n0)
                    ps = ps_proj.tile([128, 2 * H], F32, space="PSUM", tag="proj")
                    nc.tensor.matmul(out=ps[:ncols], lhsT=nfs[:, cb:cb + ncols],
                                     rhs=W[f"wsd{l}"][:], start=True, stop=True)
                    if ch < NFULL:
                        s = ch % TB
                        if s == 0:
                            stage = wpool.tile([128, TB * 2 * H], F32, tag="projst",
                                               name="projst", bufs=3)
                        nc.vector.tensor_copy(out=stage[:, s * 2 * H:(s + 1) * 2 * H],
                                              in_=ps[:])
                        if s == TB - 1:
                            c0 = (ch - s) * 128
                            tdst = T[c0: c0 + TB * 128, :].rearrange(
                                "(s p) h -> p s h", p=128)
                            nc.scalar.dma_start(
                                out=tdst,
                                in_=stage[:].rearrange("p (s h) -> p s h", s=TB))
                    else:
                        sb = wpool.tile([128, 2 * H], F32, tag="projsb", bufs=4)
                        nc.vector.tensor_copy(out=sb[:ncols], in_=ps[:ncols])
                        nc.scalar.dma_start(out=T[n0: n0 + ncols, :], in_=sb[:ncols])
                pdwin_sb = {}
                for ch in range(NWIN):
                    n0 = ch * WIN
                    ncols = min(WIN, B - n0)
                    ps = ps_proj.tile([128, 2 * H], F32, space="PSUM", tag="proj")
                    nc.tensor.matmul(out=ps[:ncols], lhsT=nft_blk[l][:, n0:n0 + ncols],
                                     rhs=W[f"wdd{l}"][:], start=True, stop=True)
                    pw = wpool.tile([WIN, 2 * H], BF16, tag=f"pdwin{ch}", name="pdwin",
                                    bufs=1)
                    nc.vector.tensor_copy(out=pw[:ncols], in_=ps[:ncols])
                    pdwin_sb[ch] = pw

                # ---- phase B/C/D: edge pipeline + per-window node MLP ----
                agg_ps = {}
                agg_sb = {}
                sidx = {}
                efs = None

                def node_mlp(w_):
                    wl = min(WIN, B - w_ * WIN)
                    sl = slice(w_ * WIN, w_ * WIN + wl)
                    ps = ps_node.tile([H, WIN], F32, space="PSUM", tag="node")
                    nc.tensor.matmul(out=ps[:, :wl], lhsT=W[f"wn_nf{l}"][:],
                                     rhs=nft_blk[l][:, sl], start=True, stop=False)
                    nc.tensor.matmul(out=ps[:, :wl], lhsT=W[f"wn_a{l}_0"][:],
                                     rhs=agg_sb[(w_, 0)][:, :wl], start=False, stop=False)
                    nc.tensor.matmul(out=ps[:, :wl], lhsT=W[f"wn_a{l}_1"][:],
                                     rhs=agg_sb[(w_, 1)][:, :wl], start=False, stop=True)
                    hsb = wpool.tile([H, WIN], BF16, tag="nodeh")
                    nc.vector.tensor_scalar(out=hsb[:, :wl], in0=ps[:, :wl],
                                            scalar1=W[f"bn1_{l}"][:], scalar2=0.0,
                                            op0=mybir.AluOpType.add,
                                            op1=mybir.AluOpType.max)
                    ps2 = ps_node.tile([H, WIN], F32, space="PSUM", tag="node")
                    nc.tensor.matmul(out=ps2[:, :wl], lhsT=W[f"wn2_{l}"][:],
                                     rhs=hsb[:, :wl], start=True, stop=True)
                    nc.vector.tensor_scalar(out=nft_blk[l + 1][:, sl], in0=ps2[:, :wl],
                                            scalar1=W[f"bn2_{l}"][:], scalar2=None,
                                            op0=mybir.AluOpType.add)
                    del agg_sb[(w_, 0)], agg_sb[(w_, 1)]

                pending = []

                def flush_scatter(ks, h1s, S_g):
                    for j in range(GROUP):
                        w_, t_ = sched[ks[j]]
                        key = (w_, t_)
                        first = key not in agg_ps
                        if first:
                            agg_ps[key] = ps_agg.tile([H + 1, 128], F32, space="PSUM",
                                                      tag="aggps", name="aggps")
                        last = (ks[j] == nch - 1) or (sched[ks[j] + 1] != key)
                        nc.tensor.matmul(
                            out=agg_ps[key][:],
                            lhsT=h1s[:, j * (H + 1): (j + 1) * (H + 1)],
                            rhs=S_g[:, j * 128: (j + 1) * 128],
                            start=first, stop=last)
                        if last:
                            asb = wpool.tile([H + 1, WIN], BF16, tag=f"aggsb{t_}", name="asb")
                            nc.vector.tensor_copy(out=asb[:], in_=agg_ps[key][:])
                            agg_sb[key] = asb
                            del agg_ps[key]
                            if t_ == NT - 1:
                                node_mlp(w_)

                for g in range(nch // GROUP):
                    ks = [g * GROUP + j for j in range(GROUP)]
                    if ks[0] % IBLK == 0:
                        k0 = ks[0]
                        kn = min(IBLK, nch - k0)
                        sidx = {
                            "gsrc": spool.tile([128, IBLK], I32, tag="gsrc", name="gsrc"),
                            "dloc": spool.tile([128, IBLK], F32, tag="dloc", name="dloc"),
                            "escl": spool.tile([128, IBLK], F32, tag="escl", name="escl"),
                        }
                        for nm in ("gsrc", "dloc", "escl"):
                            nc.sync.dma_start(out=sidx[nm][:, :kn],
                                              in_=ext[f"{nm}_cols"][:, k0:k0 + kn])
                    if ks[0] % EBLK == 0:
                        k0 = ks[0]
                        kn = min(EBLK, nch - k0)
                        efs = efpool.tile([EDGE_IN + 1, EBLK * 128], BF16, tag="efta")
                        nc.sync.dma_start(out=efs[:, :kn * 128],
                                          in_=ext["efta"][:, k0 * 128:(k0 + kn) * 128])
                        dlocr = efpool.tile([1, EBLK * 128], BF16, tag="dlocr", name="dlocr")
                        nc.sync.dma_start(out=dlocr[:, :kn * 128],
                                          in_=ext["dloc_row"][:, k0 * 128:(k0 + kn) * 128])
                    kb = ks[0] % IBLK
                    ke = ks[0] % EBLK

                    PS_g = wpool.tile([128, GROUP * 2 * H], F32, tag="psg", bufs=4)
                    efps = ps_ef.tile([128, GROUP * (H + 1)], F32, space="PSUM", tag="efps")
                    bc_ps = ps_bc.tile([128, GROUP * 128], F32, space="PSUM", tag="bc")
                    nc.tensor.matmul(out=bc_ps[:], lhsT=ones1[:],
                                     rhs=dlocr[:, ke * 128:(ke + GROUP) * 128],
                                     start=True, stop=True)
                    st_g = wpool.tile([128, GROUP * 128], BF16, tag="stg", bufs=4)
                    nc.vector.tensor_tensor(out=st_g[:], in0=iotap[:], in1=bc_ps[:],
                                            op=mybir.AluOpType.is_equal)
                    for j in range(GROUP):
                        w_, t_ = sched[ks[j]]
                        nc.gpsimd.indirect_dma_start(
                            out=PS_g[:, j * 2 * H: (j + 1) * 2 * H],
                            out_offset=None, in_=T[:],
                            in_offset=bass.IndirectOffsetOnAxis(
                                ap=sidx["gsrc"][:, kb + j: kb + j + 1], axis=0))
                        nc.tensor.matmul(
                            out=efps[:, j * (H + 1): (j + 1) * (H + 1)],
                            lhsT=efs[:, (ke + j) * 128: (ke + j + 1) * 128],
                            rhs=W[f"wef{l}_{t_}"][:], start=True, stop=False)
                        wlen_ = min(WIN, B - w_ * WIN)
                        nc.tensor.matmul(
                            out=efps[:, j * (H + 1): j * (H + 1) + H],
                            lhsT=st_g[:wlen_, j * 128: (j + 1) * 128],
                            rhs=pdwin_sb[w_][:wlen_, t_ * H:(t_ + 1) * H],
                            start=False, stop=True)

                    t1 = wpool.tile([128, GROUP * (H + 1)], F32, tag="t1", bufs=4)
                    for j in range(GROUP):
                        _, t_ = sched[ks[j]]
                        nc.vector.tensor_tensor(
                            out=t1[:, j * (H + 1): j * (H + 1) + H],
                            in0=PS_g[:, j * 2 * H + t_ * H: j * 2 * H + (t_ + 1) * H],
                            in1=efps[:, j * (H + 1): j * (H + 1) + H],
                            op=mybir.AluOpType.add)
                    nc.vector.tensor_copy(out=rg(t1[:], H + 1)[:, :, H:],
                                          in_=rg(efps[:], H + 1)[:, :, H:])
                    h1s = wpool.tile([128, GROUP * (H + 1)], BF16, tag="h1s", bufs=4)
                    for j in range(GROUP):
                        nc.vector.tensor_scalar(
                            out=h1s[:, j * (H + 1):(j + 1) * (H + 1)],
                            in0=t1[:, j * (H + 1):(j + 1) * (H + 1)],
                            scalar1=sidx["escl"][:, kb + j:kb + j + 1], scalar2=0.0,
                            op0=mybir.AluOpType.mult, op1=mybir.AluOpType.max)
                    S_g = wpool.tile([128, GROUP * 128], BF16, tag="sg", bufs=4)
                    dl_b = sidx["dloc"][:, kb:kb + GROUP].unsqueeze(2).to_broadcast(
                        [128, GROUP, 128])
                    nc.vector.tensor_tensor(out=rg(S_g[:], 128), in0=rg(iotag[:], 128),
                                            in1=dl_b, op=mybir.AluOpType.is_equal)

                    pending.append((ks, h1s, S_g))
                    if len(pending) > 1:
                        flush_scatter(*pending.pop(0))
                if pending:
                    flush_scatter(*pending.pop(0))

                if l == 0:
                    nc.sync.dma_start(out=agbounce[:], in_=nft_blk[1][:])
                    nc.gpsimd.collective_compute(
                        "AllGather", mybir.AluOpType.bypass,
                        replica_groups=[list(range(NCORES))],
                        ins=[agbounce.opt()], outs=[agout[:].opt()])
                    for c in range(NCORES):
                        nc.sync.dma_start(out=nft1d[:, c * B:(c + 1) * B],
                                          in_=agout[c])



            # ---- head ----
            NBCH = (B + 511) // 512
            for i in range(NBCH):
                c0 = i * 512
                cn = min(512, B - c0)
                cur = nft_blk[2][:, c0:c0 + cn]
                for j, (w, b) in enumerate(hd):
                    od = w.shape[1]
                    ps = ps_node.tile([H, 512], F32, space="PSUM", tag="node", name="headps")
                    nc.tensor.matmul(out=ps[:od, :cn], lhsT=W[f"wh{j}"][:], rhs=cur,
                                     start=True, stop=True)
                    if j == len(hd) - 1:
                        osb = wpool.tile([AD, 512], F32, tag="outsb")
                        nc.scalar.activation(out=osb[:, :cn], in_=ps[:od, :cn],
                                             func=AF.Tanh, bias=W[f"bh{j}"][:])
                        nc.sync.dma_start(out=out_ext[:, c0:c0 + cn], in_=osb[:, :cn])
                    else:
                        nxt = wpool.tile([H, 512], BF16, tag=f"headh{j}")
                        nc.vector.tensor_scalar(out=nxt[:, :cn], in0=ps[:od, :cn],
                                                scalar1=W[f"bh{j}"][:], scalar2=0.0,
                                                op0=mybir.AluOpType.add,
                                                op1=mybir.AluOpType.max)
                        cur = nxt[:, :cn]

    _split_multiwaits(nc)
    return nc


LAST_RESULTS = None
LAST_NC = None


def kernel(nf, ef, src, dst, etype, params, _trace=False):
    global LAST_RESULTS
    per_core, sched, nch, esched, nft0, (iotag, iotap, ones1), wts = _prep(
        nf, ef, src, dst, etype, params)
    nc = _build(sched, nch, esched, wts)

    base = dict(nft0=nft0, iotag=iotag, iotap=iotap, ones1=ones1)
    for l in (0, 1):
        base[f"wsd{l}"] = wts[l]["wsd"]
        base[f"wdd{l}"] = wts[l]["wdd"]
        for t in range(NT):
            base[f"wef{l}_{t}"] = wts[l]["wef"][t]
        base[f"wn_nf{l}"] = wts[l]["wn_nf"]
        for t in range(NT):
            base[f"wn_a{l}_{t}"] = wts[l]["wn_a"][t]
        base[f"bn1_{l}"] = wts[l]["bn1"]
        base[f"wn2_{l}"] = wts[l]["wn2"]
        base[f"bn2_{l}"] = wts[l]["bn2"]
    for i, (w, b) in enumerate(wts["head"]):
        base[f"wh{i}"] = w
        base[f"bh{i}"] = b

    in_maps = []
    for c in range(NCORES):
        m = dict(base)
        m["nft_blk0"] = per_core[c]["nft_blk"]
        for nm in ("gsrc_cols", "dloc_cols", "escl_cols", "efta"):
            m[nm] = per_core[c][nm]
        m["dloc_row"] = per_core[c]["dloc_row"]
        in_maps.append(m)

    global LAST_NC
    LAST_NC = nc
    res = run_bass_kernel_spmd(nc, in_maps, list(range(NCORES)))
    LAST_RESULTS = res
    out = np.concatenate([res.results[c]["out"].T for c in range(NCORES)], axis=0)
    return out.astype(np.float32)


# revision 31
# speedup vs baseline: 1.0384x; 1.0384x over previous
"""Trainium2 Bass kernel for a 2-layer relational GNN actor (edge MLPs per
edge type, mean aggregation by destination node, node MLP, tanh head).

Sharding: nodes are split into 8 contiguous blocks of 6250; each core owns the
edges whose dst falls in its block (host-side bucketing).  Per core, edges are
grouped by (dst window of 128 nodes, etype) and padded to 128-edge chunks so
the whole edge phase is a static instruction schedule shared by all cores.

Math (per layer, exploiting linearity of segment-sum):
  h1[e]  = Ps_t[src[e]] + Pd_t[dst[e]] + (ef[e] @ W1e_t + b1_t)   (t = etype[e])
  m[e]   = relu(h1[e]) * escale[e],   escale[e] = 1 / cnt(dst[e], t)
  agg_t[n] = sum_{e: dst=n,type=t} m[e]      (= mean of relu(h1))
  z_t[n]   = sum escale = [cnt>0]
  node in = [nf | agg_0 | z_0 | agg_1 | z_1] with the edge-MLP second layer
  (W2_t, b2_t) folded into the node-MLP first-layer weights on the host.

On device: Ps/Pd projection tables are built per layer ([4*N, 64] in DRAM) and
rows are fetched per edge chunk with indirect DMA gathers; segment-sum is a
selection-matrix matmul (S[e, j] = dst_local[e] == j) accumulating into PSUM
per (window, etype).
"""

import numpy as np

from concourse import bass, mybir
import concourse.tile as tile
from concourse.bass_utils import run_bass_kernel_spmd
from concourse.vector_clock import ScopedClock, VectorClock


# --- workarounds for a walrus build that allows at most ONE sync-wait per
# instruction: split the TileContext tail drain's waits across SP nops, and
# post-process the final module hoisting excess waits onto injected no-ops.

def _drain_and_barrier_chunked(self, tick_clock, wait_clock):
    nc = self.nc
    gclock = tick_clock.global_clock
    for lane in range(len(gclock)):
        tick = gclock[lane]
        if tick <= 0:
            continue
        vec = [0] * len(gclock)
        vec[lane] = tick
        nop_inst = nc.sync.nop(nofuse=True, hint=f"drain_wait_l{lane}")
        wait_clock.add_sem_waits(nop_inst.ins, ScopedClock({None: VectorClock(vec)}))
    nc.sync.drain()
    nc.all_engine_barrier()
    assert self.sems is not None
    popped = nc._tile_sem_poison_stack.pop()
    assert popped is self._sem_poison
    nc.clear_and_free_semaphores(list(self.sems.allocated().values()))
    nc.all_engine_barrier()


tile.TileContext._drain_and_barrier = _drain_and_barrier_chunked

_nop_counter = [0]


def _split_multiwaits(nc):
    for fn in nc.m.functions:
        for blk in fn.blocks:
            new_insts = []
            changed = False
            for inst in blk.instructions:
                si = inst.sync_info
                if si is not None and len(si.on_wait) > 1:
                    waits = list(si.on_wait)
                    for w in waits[:-1]:
                        _nop_counter[0] += 1
                        nop = mybir.InstNoOp(
                            name=f"I-waitsplit-{_nop_counter[0]}",
                            engine=inst.engine,
                            sync_info=mybir.SyncInfo(on_wait=[w], on_update=[]),
                        )
                        new_insts.append(nop)
                    si.on_wait = [waits[-1]]
                    changed = True
                new_insts.append(inst)
            if changed:
                blk.instructions = new_insts

F32 = mybir.dt.float32
BF16 = mybir.dt.bfloat16
I32 = mybir.dt.int32
AF = mybir.ActivationFunctionType

N = 50000
E = 800000
NODE_IN = 32
EDGE_IN = 16
H = 64
AD = 8
NT = 2
NCORES = 8
B = N // NCORES          # 6250 nodes per core
WIN = 128                # scatter window (nodes)
NWIN = (B + WIN - 1) // WIN   # 49
PADLOC = 300.0           # dst_local for padding edges (never matches iota)
TROWS = 4 * N            # projection table rows: src t0|src t1|dst t0|dst t1
GROUP = 4                # chunks per batched group (512 edges)
IBLK = 64                # chunks per idx/scalar stream load
EBLK = 16                # chunks per efta stream load
NCHUNK_N = (N + 127) // 128   # 391 node chunks


def _prep(nf, ef, src, dst, etype, params):
    """Host-side preprocessing: edge schedule, streams, folded weights."""
    nf = np.asarray(nf, np.float32)
    ef = np.asarray(ef, np.float32)
    src = np.asarray(src).astype(np.int32)
    dst = np.asarray(dst).astype(np.int32)
    etype = np.asarray(etype).astype(np.int32)

    core = dst // B
    win = (dst % B) // WIN

    order = np.lexsort((dst, etype, win, core))
    so_core, so_win, so_t = core[order], win[order], etype[order]
    key = (so_core.astype(np.int64) * NWIN + so_win) * NT + so_t
    cnts = np.bincount(key, minlength=NCORES * NWIN * NT).reshape(NCORES, NWIN, NT)
    chunks_per = (cnts + 127) // 128
    C = np.maximum(chunks_per.max(axis=0), 1)  # shared schedule [NWIN, NT]
    nch = int(C.sum())
    if nch % GROUP:
        C[NWIN - 1, NT - 1] += GROUP - nch % GROUP
        nch = int(C.sum())
    esched = nch * 128

    sched = []
    for w in range(NWIN):
        for t in range(NT):
            sched.extend([(w, t)] * int(C[w, t]))

    cnt_nt = np.bincount(dst * NT + etype, minlength=N * NT).astype(np.float32)

    run_starts = np.zeros(NCORES * NWIN * NT + 1, np.int64)
    np.cumsum(cnts.reshape(-1), out=run_starts[1:])

    per_core = []
    for c in range(NCORES):
        gsrc = np.zeros(esched, np.int32)
        dloc = np.full(esched, PADLOC, np.float32)
        escl = np.zeros(esched, np.float32)
        efTa = np.zeros((EDGE_IN + 1, esched), np.float32)
        efTa[EDGE_IN] = 1.0
        pos = 0
        for w in range(NWIN):
            for t in range(NT):
                ridx = (c * NWIN + w) * NT + t
                s0, s1 = run_starts[ridx], run_starts[ridx + 1]
                eidx = order[s0:s1]
                k = int(s1 - s0)
                gsrc[pos:pos + k] = src[eidx]
                dloc[pos:pos + k] = (dst[eidx] % B) % WIN
                escl[pos:pos + k] = 1.0 / cnt_nt[dst[eidx] * NT + etype[eidx]]
                efTa[:EDGE_IN, pos:pos + k] = ef[eidx].T
                pos += int(C[w, t]) * 128
        per_core.append(dict(
            gsrc_cols=np.ascontiguousarray(gsrc.reshape(nch, 128).T),
            dloc_cols=np.ascontiguousarray(dloc.reshape(nch, 128).T),
            dloc_row=np.ascontiguousarray(dloc.reshape(1, esched)),
            escl_cols=np.ascontiguousarray(escl.reshape(nch, 128).T),
            efta=np.ascontiguousarray(efTa),
            nft_blk=np.ascontiguousarray(nf[c * B:(c + 1) * B].T),
        ))

    import ml_dtypes
    bf16 = ml_dtypes.bfloat16
    for pc in per_core:
        pc["efta"] = pc["efta"].astype(bf16)
        pc["dloc_row"] = pc["dloc_row"].astype(bf16)
    nft0 = np.ascontiguousarray(nf.T)  # [32, N]
    iotag = np.tile(np.arange(128, dtype=np.float32), (128, GROUP))  # [128, 512]
    iotap = np.tile(np.arange(128, dtype=np.float32)[:, None], (1, 128 * GROUP))
    ones1 = np.ones((1, 128), bf16)

    layers = params["layers"]
    wts = {}
    for l, layer in enumerate(layers):
        nin = NODE_IN if l == 0 else H
        w1 = [np.asarray(layer["edge"][t][0][0], np.float32) for t in range(NT)]
        b1 = [np.asarray(layer["edge"][t][0][1], np.float32) for t in range(NT)]
        w2 = [np.asarray(layer["edge"][t][1][0], np.float32) for t in range(NT)]
        b2 = [np.asarray(layer["edge"][t][1][1], np.float32) for t in range(NT)]
        wn1, bn1 = (np.asarray(a, np.float32) for a in layer["node"][0])
        wn2, bn2 = (np.asarray(a, np.float32) for a in layer["node"][1])
        wsd = np.concatenate([w1[0][:nin], w1[1][:nin]], axis=1)
        wdd = np.concatenate([w1[0][nin:2 * nin], w1[1][nin:2 * nin]], axis=1)
        wef = []
        for t in range(NT):
            m = np.zeros((EDGE_IN + 1, H + 1), np.float32)
            m[:EDGE_IN, :H] = w1[t][2 * nin:]
            m[EDGE_IN, :H] = b1[t]
            m[EDGE_IN, H] = 1.0
            wef.append(m)
        wn_nf = wn1[:nin]
        wn_a = []
        for t in range(NT):
            blk = wn1[nin + t * H: nin + (t + 1) * H]
            m = np.zeros((H + 1, H), np.float32)
            m[:H] = w2[t] @ blk
            m[H] = b2[t] @ blk
            wn_a.append(m)
        wts[l] = dict(wsd=wsd, wdd=wdd,
                      wef=[m.astype(bf16) for m in wef],
                      wn_nf=wn_nf,
                      wn_a=[m.astype(bf16) for m in wn_a],
                      bn1=bn1.reshape(H, 1), wn2=wn2.astype(bf16),
                      bn2=bn2.reshape(H, 1))
    wts["head"] = [(np.asarray(w, np.float32).astype(bf16),
                    np.asarray(b, np.float32).reshape(-1, 1))
                   for w, b in params["head"]]
    return per_core, sched, nch, esched, nft0, (iotag, iotap, ones1), wts


def _build(sched, nch, esched, wts):
    nc = bass.Bass(num_devices=NCORES, dynamic_dma_scratch_size=65536)
    NIN = {0: NODE_IN, 1: H}
    hd = wts["head"]

    ext = {}

    def dram_in(name, shape, dt=F32):
        ext[name] = nc.declare_dram_parameter(name, list(shape), dt, isOutput=False)
        return ext[name]

    dram_in("nft0", [NODE_IN, N])
    dram_in("nft_blk0", [NODE_IN, B])
    dram_in("iotag", [128, 128 * GROUP])
    dram_in("iotap", [128, 128 * GROUP])
    dram_in("ones1", [1, 128], BF16)
    dram_in("gsrc_cols", [128, nch], I32)
    dram_in("dloc_cols", [128, nch])
    dram_in("dloc_row", [1, esched], BF16)
    dram_in("escl_cols", [128, nch])
    dram_in("efta", [EDGE_IN + 1, esched], BF16)
    for l in (0, 1):
        nin = NIN[l]
        dram_in(f"wsd{l}", [nin, 2 * H])
        dram_in(f"wdd{l}", [nin, 2 * H])
        for t in range(NT):
            dram_in(f"wef{l}_{t}", [EDGE_IN + 1, H + 1], BF16)
        dram_in(f"wn_nf{l}", [nin, H])
        for t in range(NT):
            dram_in(f"wn_a{l}_{t}", [H + 1, H], BF16)
        dram_in(f"bn1_{l}", [H, 1])
        dram_in(f"wn2_{l}", [H, H], BF16)
        dram_in(f"bn2_{l}", [H, 1])
    for i, (w, b) in enumerate(hd):
        dram_in(f"wh{i}", w.shape, BF16)
        dram_in(f"bh{i}", b.shape)
    out_ext = nc.declare_dram_parameter("out", [AD, B], F32, isOutput=True)

    with tile.TileContext(nc, num_cores=NCORES) as tc:
        with (
            tc.tile_pool(name="const", bufs=1) as cpool,
            tc.tile_pool(name="blk", bufs=1) as bpool,
            tc.tile_pool(name="stream", bufs=4) as spool,
            tc.tile_pool(name="efs", bufs=2) as efpool,
            tc.tile_pool(name="nfs", bufs=2) as nfspool,
            tc.tile_pool(name="work", bufs=3) as wpool,
            tc.tile_pool(name="ps_proj", bufs=2, space="PSUM") as ps_proj,
            tc.tile_pool(name="ps_ef", bufs=2, space="PSUM") as ps_ef,
            tc.tile_pool(name="ps_agg", bufs=2, space="PSUM") as ps_agg,
            tc.tile_pool(name="ps_node", bufs=1, space="PSUM") as ps_node,
            tc.tile_pool(name="ps_bc", bufs=1, space="PSUM") as ps_bc,
            tc.tile_pool(name="dram", bufs=1, space="DRAM") as dpool,
        ):
            def load(name, shape, dt=F32, pool=cpool):
                t_ = pool.tile(shape, dt, tag=name, name=name)
                nc.sync.dma_start(out=t_[:], in_=ext[name][:])
                return t_

            iotag = load("iotag", [128, 128 * GROUP])
            iotap = load("iotap", [128, 128 * GROUP])
            ones1 = load("ones1", [1, 128], BF16)
            W = {}
            for l in (0, 1):
                nin = NIN[l]
                W[f"wsd{l}"] = load(f"wsd{l}", [nin, 2 * H])
                W[f"wdd{l}"] = load(f"wdd{l}", [nin, 2 * H])
                for t in range(NT):
                    W[f"wef{l}_{t}"] = load(f"wef{l}_{t}", [EDGE_IN + 1, H + 1], BF16)
                W[f"wn_nf{l}"] = load(f"wn_nf{l}", [nin, H])
                for t in range(NT):
                    W[f"wn_a{l}_{t}"] = load(f"wn_a{l}_{t}", [H + 1, H], BF16)
                W[f"bn1_{l}"] = load(f"bn1_{l}", [H, 1])
                W[f"wn2_{l}"] = load(f"wn2_{l}", [H, H], BF16)
                W[f"bn2_{l}"] = load(f"bn2_{l}", [H, 1])
            for i, (w, b) in enumerate(hd):
                W[f"wh{i}"] = load(f"wh{i}", list(w.shape), BF16)
                W[f"bh{i}"] = load(f"bh{i}", list(b.shape))

            nb0 = bpool.tile([NODE_IN, B], F32, tag="nftblk02", name="nftblk0")
            nc.sync.dma_start(out=nb0[:], in_=ext["nft_blk0"][:])
            nft_blk = {0: nb0}
            nft_blk[1] = bpool.tile([H, B], F32, tag="nftblk1", name="nftblk1")
            nft_blk[2] = bpool.tile([H, B], BF16, tag="nftblk02", name="nftblk2")

            T = dpool.tile([N, 2 * H], F32)
            agbounce = dpool.tile([H, B], BF16)
            agout = nc.dram_tensor("agout_sh", [NCORES, H, B], BF16,
                                   addr_space="Shared")
            nft1d = dpool.tile([H, N], BF16)
            nft_src = {0: ext["nft0"], 1: nft1d}

            def rg(ap, inner):
                return ap.rearrange("p (g c) -> p g c", c=inner)

            for l in (0, 1):
                nin = NIN[l]
                # ---- phase A: projection tables into DRAM ----
                NFB = 16  # node chunks per streamed nft block
                TB = 4    # chunks batched per table-write DMA
                NFULL = (NCHUNK_N // TB) * TB if N % 128 else NCHUNK_N
                stage = None
                for ch in range(NCHUNK_N):
                    if ch % NFB == 0:
                        ncols_blk = min(NFB * 128, N - ch * 128)
                        nfs = nfspool.tile([nin, NFB * 128], F32, tag="nfs")
                        if l == 0:
                            nc.sync.dma_start(out=nfs[:, :ncols_blk],
                                              in_=nft_src[l][:, ch * 128: ch * 128 + ncols_blk])
                        else:
                            nfs16 = nfspool.tile([nin, NFB * 128], BF16, tag="nfs16",
                                                 name="nfs16")
                            nc.sync.dma_start(out=nfs16[:, :ncols_blk],
                                              in_=nft_src[l][:, ch * 128: ch * 128 + ncols_blk])
                            nc.vector.tensor_copy(out=nfs[:, :ncols_blk],
                                                  in_=nfs16[:, :ncols_blk])
                    cb = (ch % NFB) * 128
                    n0 = ch * 128
                    ncols = min(128, N - n0)
                    ps = ps_proj.tile([128, 2 * H], F32, space="PSUM", tag="proj")
                    nc.tensor.matmul(out=ps[:ncols], lhsT=nfs[:, cb:cb + ncols],
                                     rhs=W[f"wsd{l}"][:], start=True, stop=True)
                    if ch < NFULL:
                        s = ch % TB
                        if s == 0:
                            stage = wpool.tile([128, TB * 2 * H], F32, tag="projst",
                                               name="projst", bufs=3)
                        nc.vector.tensor_copy(out=stage[:, s * 2 * H:(s + 1) * 2 * H],
                                              in_=ps[:])
                        if s == TB - 1:
                            c0 = (ch - s) * 128
                            tdst = T[c0: c0 + TB * 128, :].rearrange(
                                "(s p) h -> p s h", p=128)
                            nc.scalar.dma_start(
                                out=tdst,
                                in_=stage[:].rearrange("p (s h) -> p s h", s=TB))
                    else:
                        sb = wpool.tile([128, 2 * H], F32, tag="projsb", bufs=4)
                        nc.vector.tensor_copy(out=sb[:ncols], in_=ps[:ncols])
                        nc.scalar.dma_start(out=T[n0: n0 + ncols, :], in_=sb[:ncols])
                pdwin_sb = {}
                for ch in range(NWIN):
                    n0 = ch * WIN
                    ncols = min(WIN, B - n0)
                    ps = ps_proj.tile([128, 2 * H], F32, space="PSUM", tag="proj")
                    nc.tensor.matmul(out=ps[:ncols], lhsT=nft_blk[l][:, n0:n0 + ncols],
                                     rhs=W[f"wdd{l}"][:], start=True, stop=True)
                    pw = wpool.tile([WIN, 2 * H], BF16, tag=f"pdwin{ch}", name="pdwin",
                                    bufs=1)
                    nc.vector.tensor_copy(out=pw[:ncols], in_=ps[:ncols])
                    pdwin_sb[ch] = pw

                # ---- phase B/C/D: edge pipeline + per-window node MLP ----
                agg_ps = {}
                agg_sb = {}
                sidx = {}
                efs = None

                def node_mlp(w_):
                    wl = min(WIN, B - w_ * WIN)
                    sl = slice(w_ * WIN, w_ * WIN + wl)
                    ps = ps_node.tile([H, WIN], F32, space="PSUM", tag="node")
                    nc.tensor.matmul(out=ps[:, :wl], lhsT=W[f"wn_nf{l}"][:],
                                     rhs=nft_blk[l][:, sl], start=True, stop=False)
                    nc.tensor.matmul(out=ps[:, :wl], lhsT=W[f"wn_a{l}_0"][:],
                                     rhs=agg_sb[(w_, 0)][:, :wl], start=False, stop=False)
                    nc.tensor.matmul(out=ps[:, :wl], lhsT=W[f"wn_a{l}_1"][:],
                                     rhs=agg_sb[(w_, 1)][:, :wl], start=False, stop=True)
                    hsb = wpool.tile([H, WIN], BF16, tag="nodeh")
                    nc.vector.tensor_scalar(out=hsb[:, :wl], in0=ps[:, :wl],
                                            scalar1=W[f"bn1_{l}"][:], scalar2=0.0,
                                            op0=mybir.AluOpType.add,
                                            op1=mybir.AluOpType.max)
                    ps2 = ps_node.tile([H, WIN], F32, space="PSUM", tag="node")
                    nc.tensor.matmul(out=ps2[:, :wl], lhsT=W[f"wn2_{l}"][:],
                                     rhs=hsb[:, :wl], start=True, stop=True)
                    nc.vector.tensor_scalar(out=nft_blk[l + 1][:, sl], in0=ps2[:, :wl],
                                            scalar1=W[f"bn2_{l}"][:], scalar2=None,
                                            op0=mybir.AluOpType.add)
                    del agg_sb[(w_, 0)], agg_sb[(w_, 1)]

                pending = []

                def flush_scatter(ks, h1s, S_g):
                    for j in range(GROUP):
                        w_, t_ = sched[ks[j]]
                        key = (w_, t_)
                        first = key not in agg_ps
                        if first:
                            agg_ps[key] = ps_agg.tile([H + 1, 128], F32, space="PSUM",
                                                      tag="aggps", name="aggps")
                        last = (ks[j] == nch - 1) or (sched[ks[j] + 1] != key)
                        nc.tensor.matmul(
                            out=agg_ps[key][:],
                            lhsT=h1s[:, j * (H + 1): (j + 1) * (H + 1)],
                            rhs=S_g[:, j * 128: (j + 1) * 128],
                            start=first, stop=last)
                        if last:
                            asb = wpool.tile([H + 1, WIN], BF16, tag=f"aggsb{t_}", name="asb")
                            nc.vector.tensor_copy(out=asb[:], in_=agg_ps[key][:])
                            agg_sb[key] = asb
                            del agg_ps[key]
                            if t_ == NT - 1:
                                node_mlp(w_)

                for g in range(nch // GROUP):
                    ks = [g * GROUP + j for j in range(GROUP)]
                    if ks[0] % IBLK == 0:
                        k0 = ks[0]
                        kn = min(IBLK, nch - k0)
                        sidx = {
                            "gsrc": spool.tile([128, IBLK], I32, tag="gsrc", name="gsrc"),
                            "dloc": spool.tile([128, IBLK], F32, tag="dloc", name="dloc"),
                            "escl": spool.tile([128, IBLK], F32, tag="escl", name="escl"),
                        }
                        for nm in ("gsrc", "dloc", "escl"):
                            nc.sync.dma_start(out=sidx[nm][:, :kn],
                                              in_=ext[f"{nm}_cols"][:, k0:k0 + kn])
                    if ks[0] % EBLK == 0:
                        k0 = ks[0]
                        kn = min(EBLK, nch - k0)
                        efs = efpool.tile([EDGE_IN + 1, EBLK * 128], BF16, tag="efta")
                        nc.sync.dma_start(out=efs[:, :kn * 128],
                                          in_=ext["efta"][:, k0 * 128:(k0 + kn) * 128])
                        dlocr = efpool.tile([1, EBLK * 128], BF16, tag="dlocr", name="dlocr")
                        nc.sync.dma_start(out=dlocr[:, :kn * 128],
                                          in_=ext["dloc_row"][:, k0 * 128:(k0 + kn) * 128])
                    kb = ks[0] % IBLK
                    ke = ks[0] % EBLK

                    PS_g = wpool.tile([128, GROUP * 2 * H], F32, tag="psg", bufs=4)
                    efps = ps_ef.tile([128, GROUP * (H + 1)], F32, space="PSUM", tag="efps")
                    bc_ps = ps_bc.tile([128, GROUP * 128], F32, space="PSUM", tag="bc")
                    nc.tensor.matmul(out=bc_ps[:], lhsT=ones1[:],
                                     rhs=dlocr[:, ke * 128:(ke + GROUP) * 128],
                                     start=True, stop=True)
                    st_g = wpool.tile([128, GROUP * 128], BF16, tag="stg", bufs=4)
                    nc.vector.tensor_tensor(out=st_g[:], in0=iotap[:], in1=bc_ps[:],
                                            op=mybir.AluOpType.is_equal)
                    for j in range(GROUP):
                        w_, t_ = sched[ks[j]]
                        nc.gpsimd.indirect_dma_start(
                            out=PS_g[:, j * 2 * H: (j + 1) * 2 * H],
                            out_offset=None, in_=T[:],
                            in_offset=bass.IndirectOffsetOnAxis(
                                ap=sidx["gsrc"][:, kb + j: kb + j + 1], axis=0))
                        nc.tensor.matmul(
                            out=efps[:, j * (H + 1): (j + 1) * (H + 1)],
                            lhsT=efs[:, (ke + j) * 128: (ke + j + 1) * 128],
                            rhs=W[f"wef{l}_{t_}"][:], start=True, stop=False)
                        wlen_ = min(WIN, B - w_ * WIN)
                        nc.tensor.matmul(
                            out=efps[:, j * (H + 1): j * (H + 1) + H],
                            lhsT=st_g[:wlen_, j * 128: (j + 1) * 128],
                            rhs=pdwin_sb[w_][:wlen_, t_ * H:(t_ + 1) * H],
                            start=False, stop=True)

                    t1 = wpool.tile([128, GROUP * (H + 1)], F32, tag="t1", bufs=4)
                    for j in range(GROUP):
                        _, t_ = sched[ks[j]]
                        nc.vector.tensor_tensor(
                            out=t1[:, j * (H + 1): j * (H + 1) + H],
                            in0=PS_g[:, j * 2 * H + t_ * H: j * 2 * H + (t_ + 1) * H],
                            in1=efps[:, j * (H + 1): j * (H + 1) + H],
                            op=mybir.AluOpType.add)
                    nc.vector.tensor_copy(out=rg(t1[:], H + 1)[:, :, H:],
                                          in_=rg(efps[:], H + 1)[:, :, H:])
                    h1s = wpool.tile([128, GROUP * (H + 1)], BF16, tag="h1s", bufs=4)
                    for j in range(GROUP):
                        nc.vector.tensor_scalar(
                            out=h1s[:, j * (H + 1):(j + 1) * (H + 1)],
                            in0=t1[:, j * (H + 1):(j + 1) * (H + 1)],
                            scalar1=sidx["escl"][:, kb + j:kb + j + 1], scalar2=0.0,
                            op0=mybir.AluOpType.mult, op1=mybir.AluOpType.max)
                    S_g = wpool.tile([128, GROUP * 128], BF16, tag="sg", bufs=4)
                    dl_b = sidx["dloc"][:, kb:kb + GROUP].unsqueeze(2).to_broadcast(
                        [128, GROUP, 128])
                    nc.vector.tensor_tensor(out=rg(S_g[:], 128), in0=rg(iotag[:], 128),
                                            in1=dl_b, op=mybir.AluOpType.is_equal)

                    pending.append((ks, h1s, S_g))
                    if len(pending) > 1:
                        flush_scatter(*pending.pop(0))
                if pending:
                    flush_scatter(*pending.pop(0))

                if l == 0:
                    nc.gpsimd.dma_start(out=agbounce[:], in_=nft_blk[1][:])
                    nc.gpsimd.collective_compute(
                        "AllGather", mybir.AluOpType.bypass,
                        replica_groups=[list(range(NCORES))],
                        ins=[agbounce.opt()], outs=[agout[:].opt()])
                    for c in range(NCORES):
                        nc.sync.dma_start(out=nft1d[:, c * B:(c + 1) * B],
                                          in_=agout[c])



            # ---- head ----
            NBCH = (B + 511) // 512
            for i in range(NBCH):
                c0 = i * 512
                cn = min(512, B - c0)
                cur = nft_blk[2][:, c0:c0 + cn]
                for j, (w, b) in enumerate(hd):
                    od = w.shape[1]
                    ps = ps_node.tile([H, 512], F32, space="PSUM", tag="node", name="headps")
                    nc.tensor.matmul(out=ps[:od, :cn], lhsT=W[f"wh{j}"][:], rhs=cur,
                                     start=True, stop=True)
                    if j == len(hd) - 1:
                        osb = wpool.tile([AD, 512], F32, tag="outsb")
                        nc.scalar.activation(out=osb[:, :cn], in_=ps[:od, :cn],
                                             func=AF.Tanh, bias=W[f"bh{j}"][:])
                        nc.sync.dma_start(out=out_ext[:, c0:c0 + cn], in_=osb[:, :cn])
                    else:
                        nxt = wpool.tile([H, 512], BF16, tag=f"headh{j}")
                        nc.vector.tensor_scalar(out=nxt[:, :cn], in0=ps[:od, :cn],
                                                scalar1=W[f"bh{j}"][:], scalar2=0.0,
                                                op0=mybir.AluOpType.add,
                                                op1=mybir.AluOpType.max)
                        cur = nxt[:, :cn]

    _split_multiwaits(nc)
    return nc


LAST_RESULTS = None
LAST_NC = None


def kernel(nf, ef, src, dst, etype, params, _trace=False):
    global LAST_RESULTS
    per_core, sched, nch, esched, nft0, (iotag, iotap, ones1), wts = _prep(
        nf, ef, src, dst, etype, params)
    nc = _build(sched, nch, esched, wts)

    base = dict(nft0=nft0, iotag=iotag, iotap=iotap, ones1=ones1)
    for l in (0, 1):
        base[f"wsd{l}"] = wts[l]["wsd"]
        base[f"wdd{l}"] = wts[l]["wdd"]
        for t in range(NT):
            base[f"wef{l}_{t}"] = wts[l]["wef"][t]
        base[f"wn_nf{l}"] = wts[l]["wn_nf"]
        for t in range(NT):
            base[f"wn_a{l}_{t}"] = wts[l]["wn_a"][t]
        base[f"bn1_{l}"] = wts[l]["bn1"]
        base[f"wn2_{l}"] = wts[l]["wn2"]
        base[f"bn2_{l}"] = wts[l]["bn2"]
    for i, (w, b) in enumerate(wts["head"]):
        base[f"wh{i}"] = w
        base[f"bh{i}"] = b

    in_maps = []
    for c in range(NCORES):
        m = dict(base)
        m["nft_blk0"] = per_core[c]["nft_blk"]
        for nm in ("gsrc_cols", "dloc_cols", "escl_cols", "efta"):
            m[nm] = per_core[c][nm]
        m["dloc_row"] = per_core[c]["dloc_row"]
        in_maps.append(m)

    global LAST_NC
    LAST_NC = nc
    res = run_bass_kernel_spmd(nc, in_maps, list(range(NCORES)))
    LAST_RESULTS = res
    out = np.concatenate([res.results[c]["out"].T for c in range(NCORES)], axis=0)
    return out.astype(np.float32)
